# revision 1
# baseline (speedup 1.0000x reference)
"""Performer (FAVOR+) attention TRN2 Bass kernel (v2).

Problem: B=4, N=4096, D=1024, H=16, HD=64, M=256 random features.
Sharding: 8 cores = (batch b = c//2) x (sequence half s = c%2).
Each core handles all 16 heads for 2048 query tokens and 2048 k/v
tokens; partial kv/ksum is AllReduced over seq-half pairs (split into
two collectives, heads 0:8 / 8:16, overlapped with compute).

Math simplifications (exact):
  - q-side max-over-features subtraction skipped (cancels in the
    numerator/denominator ratio; magnitudes stay well inside fp32).
  - max(normalizer, 1e-6) clamp skipped (normalizer >> 1e-6 here).
  - k-side diag term exp(CDIAG*||kh||^2 + EXP_BIAS) multiplied into
    the k features (edpe) rather than exp-biased per head, so one
    bias-free exp covers both heads of a pair via a block-diagonal
    projection matrix.
  - q-side diag & 1/sqrt(M) cancel in the out/normalizer ratio.
  - bv, bo folded host-side: out += bv @ Wo.T + bo.

Precision: q/k path fp32r; v / k' / kv / q' / o_proj path bf16
(PSUM accumulation stays fp32).
"""

import math
import sys

import numpy as np
import ml_dtypes

for _p in ("/opt/trn_rl_repo",):
    if _p not in sys.path:
        sys.path.insert(0, _p)

from concourse import bass, tile, mybir
from concourse.bass_utils import run_bass_kernel_spmd

F32 = mybir.dt.float32
F32R = mybir.dt.float32r
BF16 = mybir.dt.bfloat16

B, N, D = 4, 4096, 1024
H, HD, M = 16, 64, 256
NS = 2048  # tokens per core

SNORM = float(HD) ** -0.25
CDIAG = -0.5 * SNORM * SNORM  # -0.0625
EXP_BIAS = -0.5 * math.log(float(M))


def _build():
    nc = bass.Bass(trn_type="TRN2", target_bir_lowering=False, num_devices=8)

    qt = nc.dram_tensor("qt", [128, 8, NS], F32R, kind="ExternalInput")
    kt = nc.dram_tensor("kt", [128, 8, NS], F32R, kind="ExternalInput")
    vt = nc.dram_tensor("vt", [128, 8, NS], BF16, kind="ExternalInput")
    wqt = nc.dram_tensor("wqt", [128, 8, D], F32R, kind="ExternalInput")
    wkt = nc.dram_tensor("wkt", [128, 8, D], F32R, kind="ExternalInput")
    wvt = nc.dram_tensor("wvt", [128, 8, D], BF16, kind="ExternalInput")
    wot = nc.dram_tensor("wot", [128, 8, D], BF16, kind="ExternalInput")
    pjt = nc.dram_tensor("pjt", [HD, M], F32R, kind="ExternalInput")
    pjbdt = nc.dram_tensor("pjbdt", [128, 2 * M], F32R, kind="ExternalInput")
    mskc = nc.dram_tensor("mskc", [128, 2], F32R, kind="ExternalInput")
    onesb = nc.dram_tensor("onesb", [128, 64], BF16, kind="ExternalInput")
    oneb = nc.dram_tensor("oneb", [128, 16, 16, 1], BF16, kind="ExternalInput")
    bqc = nc.dram_tensor("bqc", [128, 8], F32, kind="ExternalInput")
    bkc = nc.dram_tensor("bkc", [128, 8], F32, kind="ExternalInput")
    out = nc.dram_tensor("out", [NS, D], F32, kind="ExternalOutput")

    mult = mybir.AluOpType.mult
    Exp = mybir.ActivationFunctionType.Exp

    with tile.TileContext(nc) as tc:
        # ---------------- persistent constants ----------------
        pj2, free_pj2 = tc.tile([128, M], F32R, name="pj2")      # proj.T * s, both halves
        pjbd, free_pjbd = tc.tile([128, 2 * M], F32R, name="pjbd")  # block-diag
        msk2, free_msk2 = tc.tile([128, 2], F32R, name="msk2")
        ones64, free_ones64 = tc.tile([128, 64], BF16, name="ones64")
        bq_sb, free_bq = tc.tile([128, 8], F32, name="bq_sb")
        bk_sb, free_bk = tc.tile([128, 8], F32, name="bk_sb")
        ebias, free_ebias = tc.tile([128, 1], F32, name="ebias")

        nc.sync.dma_start(pj2[0:64, :], pjt[:, :])
        nc.sync.dma_start(pj2[64:128, :], pjt[:, :])
        nc.sync.dma_start(pjbd[:, :], pjbdt[:, :])
        nc.sync.dma_start(msk2[:, :], mskc[:, :])
        nc.sync.dma_start(ones64[:, :], onesb[:, :])
        nc.sync.dma_start(bq_sb[:, :], bqc[:, :])
        nc.sync.dma_start(bk_sb[:, :], bkc[:, :])
        nc.vector.memset(ebias[:, :], EXP_BIAS)

        # ---------------- big staging buffers ----------------
        # khT[p, dc, t] = kh[t, dc*128+p]  (head h=2*dc+(p>=64))
        khT, free_khT = tc.tile([128, 8, NS], F32R, name="khT")
        # v_nat[p, tcc, h, 0:64] = vh[tcc*128+p, h*64+d]; col 64 = 1.0
        v_nat, free_v_nat = tc.tile([128, 16, 16, 65], BF16, name="v_nat")
        # edpe[p, pair, tcc, hh] = exp(CDIAG*||kh||^2 + EXP_BIAS)
        edpe, free_edpe = tc.tile([128, 8, 16, 2], F32, name="edpe")

        nc.sync.dma_start(v_nat[:, :, :, 64:65], oneb[:, :, :, :])

        # ---------------- phase 1k: khT = Wk @ k.T + bk ----------------
        with tc.tile_pool(name="wk", bufs=1) as wkpool, \
             tc.tile_pool(name="kin", bufs=2) as kinpool, \
             tc.tile_pool(name="pk", bufs=2, space="PSUM") as pkpool:
            wk_sb = wkpool.tile([128, 8, D], F32R, name="wk_sb")
            _qeng0 = [nc.sync, nc.scalar]
            for _ic in range(8):
                _qeng0[_ic % 2].dma_start(wk_sb[:, _ic, :], wkt[:, _ic, :])
            for tcc in range(4):
                kt_in = kinpool.tile([128, 8, 512], F32R, name="kt_in")
                _qeng0[(tcc + 1) % 2].dma_start(
                    kt_in[:, :, :], kt[:, :, tcc * 512:(tcc + 1) * 512])
                for dc in range(8):
                    ps = pkpool.tile([128, 512], F32, name="pk")
                    for ic in range(8):
                        nc.tensor.matmul(
                            ps[:, :],
                            wk_sb[:, ic, dc * 128:(dc + 1) * 128],
                            kt_in[:, ic, :],
                            start=(ic == 0), stop=(ic == 7),
                        )
                    # bias add on ACT (idle in this phase); DVE does squares.
                    nc.scalar.add(khT[:, dc, tcc * 512:(tcc + 1) * 512],
                                  ps[:, :], bk_sb[:, dc:dc + 1])

        # ---------------- phase 1v + A2 interleaved ----------------
        # A2: dpe[p,pair,tcc,hh] = CDIAG*||kh||^2 (via masked matmul), then
        # edpe = exp(dpe + EXP_BIAS) in one activation.
        with tc.tile_pool(name="wv", bufs=1) as wvpool, \
             tc.tile_pool(name="vin", bufs=2) as vinpool, \
             tc.tile_pool(name="sq", bufs=3) as sqpool, \
             tc.tile_pool(name="pdp", bufs=1, space="PSUM") as pdppool, \
             tc.tile_pool(name="pv", bufs=2, space="PSUM") as pvpool:
            wv_sb = wvpool.tile([128, 8, D], BF16, name="wv_sb")
            _qeng1 = [nc.sync, nc.scalar]
            for _ic in range(8):
                _qeng1[_ic % 2].dma_start(wv_sb[:, _ic, :], wvt[:, _ic, :])
            dps = pdppool.tile([128, 8, 16, 2], F32, name="dps")
            for tcc in range(16):
                vt_in = vinpool.tile([128, 8, 128], BF16, name="vt_in")
                _qeng1[(tcc + 1) % 2].dma_start(
                    vt_in[:, :, :], vt[:, :, tcc * 128:(tcc + 1) * 128])
                for dc in range(2):
                    ps = pvpool.tile([128, 8, 64], F32, name="pv")
                    for ic in range(8):
                        nc.tensor.matmul(
                            ps[:, :, :],
                            vt_in[:, ic, :],
                            wv_sb[:, ic, dc * 512:(dc + 1) * 512],
                            start=(ic == 0), stop=(ic == 7),
                        )
                    nc.vector.tensor_copy(v_nat[:, tcc, dc * 8:(dc + 1) * 8, 0:64],
                                          ps[:, :, :])
                if tcc % 2 == 0:
                    pair = tcc // 2
                    for t4 in range(4):
                        src = khT[:, pair, t4 * 512:(t4 + 1) * 512]
                        sq = sqpool.tile([128, 512], F32R, name="sq")
                        nc.vector.tensor_tensor(sq[:, :], src, src, mult)
                        for c in range(4):
                            nc.tensor.matmul(
                                dps[:, pair, t4 * 4 + c, :],
                                sq[:, c * 128:(c + 1) * 128],
                                msk2[:, :],
                                start=True, stop=True)
            nc.scalar.activation(edpe[:, :, :, :], dps[:, :, :, :], Exp,
                                 bias=ebias[:, 0:1])

        # ---------------- collectives DRAM staging ----------------
        with tc.tile_pool(name="dramb", bufs=4, space="DRAM") as dramb:
            cin1 = dramb.tile([128, 8, 2, 65], F32, name="cin1")
            cout1 = dramb.tile([128, 8, 2, 65], F32, name="cout1")
            cin2 = dramb.tile([128, 8, 2, 65], F32, name="cin2")
            cout2 = dramb.tile([128, 8, 2, 65], F32, name="cout2")
            kvA, free_kvA = tc.tile([128, 8, 2, 65], F32, name="kvA")
            kvB, free_kvB = tc.tile([128, 8, 2, 65], F32, name="kvB")

            # ---------------- stage B: k features + kvT accumulation -----
            # kf[t, 0:256 | 256:512] = (khT_pair)^T @ [pj|0 ; 0|pj]
            # k2 = exp(kf) * edpe (bf16); kvT[m, hd65] = sum_t k2 v_nat
            with tc.tile_pool(name="pkf", bufs=3, space="PSUM") as pkfpool, \
                 tc.tile_pool(name="k2", bufs=3) as k2pool, \
                 tc.tile_pool(name="pkv", bufs=2, space="PSUM") as pkvpool:
                for pair in range(8):
                    kvp = pkvpool.tile([128, 2, 2, 65], F32, name="kvp")
                    for tcc in range(16):
                        kf = pkfpool.tile([128, 512], F32, name="kf")
                        nc.tensor.matmul(kf[:, :],
                                         khT[:, pair, tcc * 128:(tcc + 1) * 128],
                                         pjbd[:, :], start=True, stop=True)
                        k2 = k2pool.tile([128, 512], BF16, name="k2")
                        nc.scalar.activation(k2[:, :], kf[:, :], Exp)
                        nc.vector.tensor_scalar_mul(
                            k2[:, 0:256], k2[:, 0:256], edpe[:, pair, tcc, 0:1])
                        nc.vector.tensor_scalar_mul(
                            k2[:, 256:512], k2[:, 256:512], edpe[:, pair, tcc, 1:2])
                        # kvp's 4 slices share one PSUM bank = one zero
                        # region: a single accumulation group. start marks
                        # the whole region pending-zero, so only the first
                        # matmul starts and only the last stops.
                        for hh in range(2):
                            h = 2 * pair + hh
                            for fc in range(2):
                                nc.tensor.matmul(
                                    kvp[:, hh, fc, :],
                                    k2[:, hh * 256 + fc * 128:hh * 256 + (fc + 1) * 128],
                                    v_nat[:, tcc, h, 0:65],
                                    start=(tcc == 0 and hh == 0 and fc == 0),
                                    stop=(tcc == 15 and hh == 1 and fc == 1),
                                )
                    if pair < 4:
                        nc.vector.tensor_copy(kvA[:, 2 * (pair % 4):2 * (pair % 4) + 2, :, :],
                                              kvp[:, :, :, :])
                    else:
                        nc.vector.tensor_copy(kvB[:, 2 * (pair % 4):2 * (pair % 4) + 2, :, :],
                                              kvp[:, :, :, :])
                    if pair == 3:
                        nc.gpsimd.dma_start(cin1[:, :, :, :], kvA[:, :, :, :])
                        nc.gpsimd.collective_compute(
                            "AllReduce", mybir.AluOpType.add,
                            replica_groups=[[0, 1], [2, 3], [4, 5], [6, 7]],
                            ins=[cin1.opt()], outs=[cout1.opt()],
                        )
                    if pair == 7:
                        nc.gpsimd.dma_start(cin2[:, :, :, :], kvB[:, :, :, :])
                        nc.gpsimd.collective_compute(
                            "AllReduce", mybir.AluOpType.add,
                            replica_groups=[[0, 1], [2, 3], [4, 5], [6, 7]],
                            ins=[cin2.opt()], outs=[cout2.opt()],
                        )

            # khT / v_nat / edpe no longer needed; reuse for qhT (LIFO order).
            free_kvB()
            free_kvA()
            free_edpe()
            free_v_nat()
            free_khT()

            # ---------------- phase 1q: qhT = Wq @ q.T + bq (overlaps cc) --
            qhT, free_qhT = tc.tile([128, 8, NS], F32R, name="qhT")
            kv_nat, free_kv_nat = tc.tile([128, 16, 2, 65], BF16, name="kv_nat")
            krep, free_krep = tc.tile([128, 16, 2, 64], BF16, name="krep")
            kvf32, free_kvf32 = tc.tile([128, 16, 2, 65], F32, name="kvf32")
            with tc.tile_pool(name="wq", bufs=1) as wqpool, \
                 tc.tile_pool(name="qin", bufs=2) as qinpool, \
                 tc.tile_pool(name="pq", bufs=2, space="PSUM") as pqpool:
                wq_sb = wqpool.tile([128, 8, D], F32R, name="wq_sb")
                # split the weight load across two engine DMA queues so 1q
                # isn't gated on one serial queue after stage B (gpsimd's
                # queue is blocked behind the collectives).
                _qeng = [nc.sync, nc.scalar]
                for _ic in range(8):
                    _qeng[_ic % 2].dma_start(wq_sb[:, _ic, :], wqt[:, _ic, :])
                for tcc in range(4):
                    qt_in = qinpool.tile([128, 8, 512], F32R, name="qt_in")
                    _qeng[(tcc + 1) % 2].dma_start(
                        qt_in[:, :, :], qt[:, :, tcc * 512:(tcc + 1) * 512])
                    for dc in range(8):
                        ps = pqpool.tile([128, 512], F32, name="pq")
                        for ic in range(8):
                            nc.tensor.matmul(
                                ps[:, :],
                                wq_sb[:, ic, dc * 128:(dc + 1) * 128],
                                qt_in[:, ic, :],
                                start=(ic == 0), stop=(ic == 7),
                            )
                        nc.scalar.add(qhT[:, dc, tcc * 512:(tcc + 1) * 512],
                                      ps[:, :], bq_sb[:, dc:dc + 1])

                # ------------ kv gather + bf16 cast + ksum replicate --
                nc.sync.dma_start(kvf32[:, 0:8, :, :], cout1[:, :, :, :])
                nc.sync.dma_start(kvf32[:, 8:16, :, :], cout2[:, :, :, :])
                nc.vector.tensor_copy(kv_nat[:, 0:8, :, :],
                                      kvf32[:, 0:8, :, :])
                nc.vector.tensor_copy(kv_nat[:, 8:16, :, :],
                                      kvf32[:, 8:16, :, :])
                for h in range(16):
                    for fc in range(2):
                        nc.vector.tensor_scalar_mul(
                            krep[:, h, fc, :], ones64[:, :],
                            kvf32[:, h, fc, 64:65])

            # ---------------- stage E: q features, attention, o_proj --
            with tc.tile_pool(name="wo", bufs=1) as wopool, \
                 tc.tile_pool(name="attn", bufs=2) as attnpool, \
                 tc.tile_pool(name="aodd", bufs=2) as oddpool, \
                 tc.tile_pool(name="qp", bufs=4) as qppool, \
                 tc.tile_pool(name="rec", bufs=4) as recpool, \
                 tc.tile_pool(name="osb", bufs=2) as osbpool, \
                 tc.tile_pool(name="pqf", bufs=2, space="PSUM") as pqfpool, \
                 tc.tile_pool(name="ppo", bufs=2, space="PSUM") as ppopool, \
                 tc.tile_pool(name="po5", bufs=2, space="PSUM") as po5pool:
                    wo_sb = wopool.tile([128, 8, D], BF16, name="wo_sb")
                    for _ic in range(8):
                        nc.sync.dma_start(wo_sb[:, _ic, :], wot[:, _ic, :])

                    def emit_oproj(src_tc4, attn_t, tcc, j):
                        # one o_proj output block [128 tokens x 512 dout]
                        p5 = po5pool.tile([128, 512], F32, name="p5")
                        for pair in range(8):
                            nc.tensor.matmul(
                                p5[:, :],
                                attn_t[:, pair, tcc * 128:(tcc + 1) * 128],
                                wo_sb[:, pair, j * 512:(j + 1) * 512],
                                start=(pair == 0), stop=(pair == 7),
                            )
                        o_sb = osbpool.tile([128, 512], F32, name="o_sb")
                        nc.vector.tensor_copy(o_sb[:, :], p5[:, :])
                        nc.sync.dma_start(
                            out[src_tc4 * 512 + tcc * 128:
                                src_tc4 * 512 + (tcc + 1) * 128,
                                j * 512:(j + 1) * 512],
                            o_sb[:, :])

                    prev_attn = None
                    for tc4 in range(4):
                        tsl = slice(tc4 * 512, (tc4 + 1) * 512)
                        attn = attnpool.tile([128, 8, 512], BF16, name="attn")
                        aodd = oddpool.tile([128, 8, 512], BF16, name="aodd")
                        _og = 0
                        for hh in range(2):
                            base = hh * 64
                            for pg in range(4):
                                pos = []
                                for pj in range(2):
                                    pos.append(ppopool.tile([64, 2, 512], F32,
                                                            name="po"))
                                for fc in range(2):
                                    for pj in range(2):
                                        pair = 2 * pg + pj
                                        h = 2 * pair + hh
                                        qf = pqfpool.tile([128, 512], F32, name="qf")
                                        nc.tensor.matmul(
                                            qf[:, :],
                                            pj2[base:base + 64, fc * 128:(fc + 1) * 128],
                                            qhT[base:base + 64, pair, tsl],
                                            start=True, stop=True)
                                        qp = qppool.tile([128, 512], BF16, name="qp")
                                        nc.scalar.activation(qp[:, :], qf[:, :], Exp)
                                        nc.tensor.matmul(
                                            pos[pj][:, 0, :],
                                            kv_nat[:, h, fc, 0:64],
                                            qp[:, :],
                                            start=(fc == 0), stop=(fc == 1))
                                        nc.tensor.matmul(
                                            pos[pj][:, 1, :],
                                            krep[:, h, fc, :],
                                            qp[:, :],
                                            start=(fc == 0), stop=(fc == 1))
                                for pj in range(2):
                                    pair = 2 * pg + pj
                                    # 1/norm via exp(-ln(norm)) on ACT: DVE's
                                    # exact reciprocal is ~6.5ns/elem (213us
                                    # total); both Ln and Exp live in the
                                    # natural_log_exp_and_others table so no
                                    # table reloads.
                                    nln = recpool.tile([64, 512], F32,
                                                       name="nln")
                                    nc.scalar.activation(
                                        nln[:, :], pos[pj][:, 1, :],
                                        mybir.ActivationFunctionType.Ln)
                                    rec = recpool.tile([64, 512], BF16,
                                                       name="rec")
                                    nc.scalar.activation(
                                        rec[:, :], nln[:, :],
                                        Exp, scale=-1.0)
                                    if hh == 0:
                                        nc.vector.tensor_tensor(
                                            attn[0:64, pair, :],
                                            pos[pj][:, 0, :], rec[:, :], mult)
                                    else:
                                        nc.vector.tensor_tensor(
                                            aodd[0:64, pair, :],
                                            pos[pj][:, 0, :], rec[:, :], mult)
                                        nc.sync.dma_start(attn[64:128, pair, :],
                                                          aodd[0:64, pair, :])
                                # software-pipeline o_proj: interleave the
                                # previous tc4's 8 output blocks between this
                                # tc4's 8 (hh, pg) groups so its PE time hides
                                # under the ACT-bound feature work.
                                if prev_attn is not None:
                                    emit_oproj(tc4 - 1, prev_attn,
                                               _og // 2, _og % 2)
                                    _og += 1
                        prev_attn = attn
                    for tcc in range(4):
                        for j in range(2):
                            emit_oproj(3, prev_attn, tcc, j)

            for f in (free_kvf32, free_krep, free_kv_nat, free_qhT,):
                f()

        for f in (free_ebias, free_bk, free_bq, free_ones64, free_msk2,
                  free_pjbd, free_pj2):
            f()

    # TRN2 walrus codegen allows at most 1 sync wait per instruction
    # (2 on InstEventSemaphore); split excess waits into event semaphores.
    import bass_rust
    bass_rust.generate_event_semaphores(nc)
    return nc


_CACHE = {}


def _get_nc():
    if "nc" not in _CACHE:
        _CACHE["nc"] = _build()
    return _CACHE["nc"]


def _shard(x, dtype=np.float32):
    # [2048, 1024] token-slice -> [128, 8, 2048] with [p, ic, t] = x[t, ic*128+p]
    return np.ascontiguousarray(
        x.T.reshape(8, 128, NS).transpose(1, 0, 2)).astype(dtype)


def _wlayout(w, dtype=np.float32):
    # W [D, D] -> [128, 8, D] with [p, ic, d] = W[d, ic*128+p]
    return np.ascontiguousarray(
        w.T.reshape(8, 128, D).transpose(1, 0, 2)).astype(dtype)


def _run(nc, in_maps, trace=False, tmpdir=None):
    return run_bass_kernel_spmd(nc, in_maps, list(range(8)), trace=trace,
                                tmpdir=tmpdir)


def _host_inputs(q, k, v, Wq, bq, Wk, bk, Wv, bv, Wo, bo, proj):
    bf16 = ml_dtypes.bfloat16

    pjs = (proj.T * SNORM).astype(np.float32)          # [64, 256]
    pjt = np.ascontiguousarray(pjs)
    pjbdt = np.zeros((128, 2 * M), dtype=np.float32)   # block-diag
    pjbdt[0:64, 0:M] = pjs
    pjbdt[64:128, M:2 * M] = pjs
    wqt = _wlayout(Wq)
    wkt = _wlayout(Wk)
    wvt = _wlayout(Wv, bf16)
    wot = _wlayout(Wo, bf16)
    bqc = np.ascontiguousarray(bq.reshape(8, 128).T).astype(np.float32)
    bkc = np.ascontiguousarray(bk.reshape(8, 128).T).astype(np.float32)
    mskc = np.zeros((128, 2), dtype=np.float32)
    mskc[0:64, 0] = CDIAG
    mskc[64:128, 1] = CDIAG
    onesb = np.ones((128, 64), dtype=bf16)
    oneb = np.ones((128, 16, 16, 1), dtype=bf16)

    in_maps = []
    for c in range(8):
        b, s = divmod(c, 2)
        sl = slice(s * NS, (s + 1) * NS)
        in_maps.append({
            "qt": _shard(q[b, sl, :]),
            "kt": _shard(k[b, sl, :]),
            "vt": _shard(v[b, sl, :], bf16),
            "wqt": wqt, "wkt": wkt, "wvt": wvt, "wot": wot,
            "pjt": pjt, "pjbdt": pjbdt, "bqc": bqc, "bkc": bkc,
            "mskc": mskc, "onesb": onesb, "oneb": oneb,
        })
    return in_maps


def kernel(q, k, v, Wq, bq, Wk, bk, Wv, bv, Wo, bo, proj,
           _trace=False, _tmpdir=None):
    nc = _get_nc()
    in_maps = _host_inputs(q, k, v, Wq, bq, Wk, bk, Wv, bv, Wo, bo, proj)

    res = _run(nc, in_maps, trace=_trace, tmpdir=_tmpdir)

    bo_eff = (bv @ Wo.T + bo).astype(np.float32)
    full = np.empty((B, N, D), dtype=np.float32)
    for c in range(8):
        b, s = divmod(c, 2)
        full[b, s * NS:(s + 1) * NS, :] = res.results[c]["out"] + bo_eff

    if _trace:
        return full, res
    return full

